# revision 20
# baseline (speedup 1.0000x reference)
"""CrossScanAttention (bimamba-v3) Trainium2 kernel.

Full inputs -> shard batch across 8 NeuronCores (2 batches/core) -> full output.
Self-contained: hardcodes all shapes; no sibling imports, no file reads.

Per-core pipeline (v2):
  pool4 (DVE multi-axis reduce) -> pre-proj+LN (PE + bn_stats, ln gamma/beta
  folded into in_proj weights) -> in_proj (PE, bias via ACT) ->
  D1 sweep (all 6 units): causal dwconv (DVE), silu (ACT, one table set),
    x_proj (PE), B/C rows bounced to DRAM;
  D2 sweep: dt_proj (PE), softplus (ACT exp+ln), dA = exp(A_s*dt) via 16 ACT
    exps (natural_log_exp set resident for the whole sweep), u = dt*xs (DVE);
  D3 sweep: B broadcast (DRAM bounce DMA), dBu (DVE), hardware
    tensor_tensor_scan in two 8-state halves, C broadcast, y = C-weighted
    reduce (tree levels 1-2 on GPSIMD, 3-4 on DVE), silu(z) gate, 3-branch sum;
  folded out_proj+post head (PE dot + ACT sigmoid).
"""

import numpy as np

# ---- problem constants ----
B, C, H, W = 16, 768, 32, 32
D_MODEL, D_INNER, D_STATE, DT_RANK, D_CONV = 64, 128, 16, 4, 4
LN_EPS = 1e-5
NCORES = 8
BL = B // NCORES          # 2 local batches per core
L = C                     # 768
TS = L + 2                # 770: segment with 2 zero break-cols (keeps 4B align)
SH = 8                    # states per scan half

USE_GPSIMD_TREE = False

_cached = {}

# p128f column layout
PF_A, PF_CW, PF_CB, PF_DTB, PF_D, PF_V, PF_BE = 0, 48, 60, 63, 66, 69, 70
PF_NCOL = 76


def _build_nc():
    import concourse.bass as bass
    import concourse.bacc as bacc
    import concourse.tile as tile
    import concourse.mybir as mybir
    from concourse.masks import make_identity
    from contextlib import ExitStack

    f32 = mybir.dt.float32
    bf16 = mybir.dt.bfloat16
    AL = mybir.AluOpType
    AF = mybir.ActivationFunctionType
    AX = mybir.AxisListType

    nc = bacc.Bacc("TRN2", target_bir_lowering=False, debug=False)

    img1 = nc.dram_tensor("img1", (BL, C, H, W), f32, kind="ExternalInput").ap()
    img2 = nc.dram_tensor("img2", (BL, C, H, W), f32, kind="ExternalInput").ap()
    p128f = nc.dram_tensor("p128f", (128, PF_NCOL), f32, kind="ExternalInput").ap()
    p128b = nc.dram_tensor("p128b", (128, 108), bf16, kind="ExternalInput").ap()
    p17f = nc.dram_tensor("p17f", (17, 64), bf16, kind="ExternalInput").ap()
    p4b = nc.dram_tensor("p4b", (4, 384), bf16, kind="ExternalInput").ap()
    p64b = nc.dram_tensor("p64b", (64, 512), bf16, kind="ExternalInput").ap()
    miscf = nc.dram_tensor("miscf", (1, 2), f32, kind="ExternalInput").ap()
    att_out = nc.dram_tensor("att", (1, BL * L), bf16, kind="ExternalOutput").ap()

    def rev(ap):
        """View with the last free dim reversed."""
        steps = [list(x) for x in ap.ap]
        st, n = steps[-1]
        newap = steps[:-1] + [[-st, n]]
        return bass.AP(tensor=ap.tensor, offset=ap.offset + st * (n - 1), ap=newap)

    def units():
        # match stage C xpad creation order: (b, br01) for i=0, then br2
        for b in range(BL):
            for br in (0, 1):
                yield b, br
        for b in range(BL):
            yield b, 2

    with nc.allow_low_precision("bf16 intermediate precision is sufficient"), \
         tile.TileContext(nc) as tc, ExitStack() as ctx:
        consts = ctx.enter_context(tc.tile_pool(name="consts", bufs=1))
        imgp = ctx.enter_context(tc.tile_pool(name="imgp", bufs=2))
        small_ps = ctx.enter_context(tc.tile_pool(name="small_ps", bufs=4, space="PSUM"))
        mm_ps = ctx.enter_context(tc.tile_pool(name="mm_ps", bufs=2, space="PSUM"))
        stats = ctx.enter_context(tc.tile_pool(name="stats", bufs=4))
        xtp = ctx.enter_context(tc.tile_pool(name="xtp", bufs=3))
        szp = ctx.enter_context(tc.tile_pool(name="szp", bufs=4))
        xpadp = ctx.enter_context(tc.tile_pool(name="xpadp", bufs=3))
        ysp = ctx.enter_context(tc.tile_pool(name="ysp", bufs=2))
        dbcp = ctx.enter_context(tc.tile_pool(name="dbcp", bufs=6))
        xsp = ctx.enter_context(tc.tile_pool(name="xsp", bufs=6))
        up = ctx.enter_context(tc.tile_pool(name="up", bufs=3))
        seqp = ctx.enter_context(tc.tile_pool(name="seqp", bufs=2))
        dtp = ctx.enter_context(tc.tile_pool(name="dtp", bufs=4))
        dAp = ctx.enter_context(tc.tile_pool(name="dAp", bufs=4))
        dBup = ctx.enter_context(tc.tile_pool(name="dBup", bufs=2))
        hp = ctx.enter_context(tc.tile_pool(name="hp", bufs=3))
        bcp = ctx.enter_context(tc.tile_pool(name="bcp", bufs=2))
        dramp = ctx.enter_context(tc.tile_pool(name="dramp", bufs=6, space="DRAM"))
        outp = ctx.enter_context(tc.tile_pool(name="outp", bufs=1))

        # ---- constants ----
        c128f = consts.tile([128, PF_NCOL], f32)
        nc.sync.dma_start(out=c128f, in_=p128f)
        c128b = consts.tile([128, 108], bf16)
        nc.sync.dma_start(out=c128b, in_=p128b)
        c17 = consts.tile([16, 64], bf16)
        nc.sync.dma_start(out=c17, in_=p17f[0:16, :])
        c_preb = consts.tile([1, 64], bf16)
        nc.sync.dma_start(out=c_preb, in_=p17f[16:17, :])
        ones1 = consts.tile([1, 128], bf16)
        nc.vector.memset(ones1[:, :], 1.0)
        c4 = consts.tile([4, 384], bf16)
        nc.sync.dma_start(out=c4, in_=p4b)
        c64 = consts.tile([64, 512], bf16)
        nc.sync.dma_start(out=c64, in_=p64b)
        cmisc = consts.tile([1, 2], f32)
        nc.sync.dma_start(out=cmisc, in_=miscf)
        identb = consts.tile([128, 128], bf16)
        make_identity(nc, identb[:, :])
        vcol_b = consts.tile([128, 1], bf16)
        nc.scalar.copy(out=vcol_b[:, :], in_=c128f[:, PF_V:PF_V + 1])

        # ---- stage A: pool -> pooledT [16, 4*768] bf16 ----
        pooledT = outp.tile([16, 4 * L], bf16, tag="big")
        imgs = [img1, img2]
        for i in range(2):
            for b in range(BL):
                for k in range(6):
                    it = imgp.tile([128, 1024], f32, tag="imgtile")
                    src = imgs[i][b, k * 128:(k + 1) * 128, :, :].rearrange(
                        "c h w -> c (h w)")
                    nc.sync.dma_start(out=it, in_=src)
                    v5 = it[:, :].rearrange("p (jr hb jc wb) -> p jr jc hb wb",
                                            jr=4, hb=8, jc=4, wb=8)
                    psum = stats.tile([128, 16], bf16, tag="poolsum")
                    nc.vector.tensor_reduce(out=psum, in_=v5, axis=AX.XY, op=AL.add)
                    pt_ps = small_ps.tile([16, 128], bf16, tag="sm")
                    nc.tensor.transpose(pt_ps[:, :], psum[:, :], identb[:, :])
                    col = (i * BL + b) * L + k * 128
                    nc.scalar.copy(out=pooledT[0:16, col:col + 128], in_=pt_ps)

        # ---- stage B: pre-proj + LN -> xT[(i,b)] [64, 768] bf16 ----
        xT = {}
        for i in range(2):
            for b in range(BL):
                xt = xtp.tile([64, L], bf16, tag="xT")
                for k in range(6):
                    col = (i * BL + b) * L + k * 128
                    xp_ps = small_ps.tile([128, 64], f32, tag="sm")
                    nc.tensor.matmul(xp_ps[:, :], lhsT=pooledT[:, col:col + 128],
                                     rhs=c17[:, :], start=True, stop=False)
                    nc.tensor.matmul(xp_ps[:, :], lhsT=ones1[:, :],
                                     rhs=c_preb[:, :], start=False, stop=True)
                    st6 = stats.tile([128, 6], f32, tag="bnst")
                    nc.vector.bn_stats(out=st6, in_=xp_ps)
                    mv = stats.tile([128, 2], f32, tag="bnmv")
                    nc.vector.bn_aggr(out=mv, in_=st6)
                    sq = stats.tile([128, 1], f32, tag="sq")
                    nc.scalar.activation(sq[:, :], mv[:, 1:2], AF.Sqrt,
                                         bias=c128f[:, 74:75])
                    rs = stats.tile([128, 1], f32, tag="rs")
                    nc.vector.reciprocal(out=rs[:, :], in_=sq[:, :])
                    xn = stats.tile([128, 64], bf16, tag="xn")
                    nc.vector.tensor_scalar(out=xn[:, :], in0=xp_ps[:, :],
                                            scalar1=mv[:, 0:1], scalar2=rs[:, 0:1],
                                            op0=AL.subtract, op1=AL.mult)
                    xn_ps = small_ps.tile([64, 128], bf16, tag="sm")
                    nc.tensor.transpose(xn_ps[:, :], xn[:, :], identb[:, :])
                    nc.vector.tensor_copy(out=xt[:, k * 128:(k + 1) * 128],
                                          in_=xn_ps[:, :])
                xT[(i, b)] = xt

        # ---- stage C: in_proj -> xpad[(b,br)] [128, 771] bf16, sz[(i,b)] ----
        xpad = {}
        sz = {}
        for i in range(2):
            for b in range(BL):
                for h in range(2):
                    ps = mm_ps.tile([128, L], f32, tag="mm")
                    wsl = c64[:, i * 256 + h * 128: i * 256 + (h + 1) * 128]
                    nc.tensor.matmul(ps[:, 0:512], lhsT=wsl, rhs=xT[(i, b)][:, 0:512],
                                     start=True, stop=True)
                    nc.tensor.matmul(ps[:, 512:768], lhsT=wsl,
                                     rhs=xT[(i, b)][:, 512:768], start=True, stop=True)
                    be = c128f[:, PF_BE + i * 2 + h: PF_BE + i * 2 + h + 1]
                    if h == 0:
                        brs = [0, 1] if i == 0 else [2]
                        for br in brs:
                            xp = xpadp.tile([128, L + 3], bf16, tag="xpad")
                            nc.vector.memset(xp[:, 0:3], 0.0)
                            src = ps[:, :] if br != 1 else rev(ps[:, :])
                            nc.vector.tensor_scalar_add(xp[:, 3:L + 3], src, be)
                            xpad[(b, br)] = xp
                    else:
                        z = szp.tile([128, L], bf16, tag="sz")
                        nc.scalar.activation(z[:, :], ps[:, :], AF.Silu, bias=be)
                        sz[(i, b)] = z

        # ---- D1 sweep: conv + silu + xproj + B/C bounce (Silu set resident) ----
        xs_map = {}
        dbc_map = {}
        bc_dram = {}
        for b, br in units():
            xp = xpad[(b, br)]
            cw = c128f[:, PF_CW + br * 4: PF_CW + (br + 1) * 4]
            cacc = seqp.tile([128, L], bf16, tag="cacc")
            nc.vector.tensor_scalar_mul(cacc[:, :], xp[:, 0:L], cw[:, 0:1])
            for k in range(1, 4):
                ctap = seqp.tile([128, L], bf16, tag="ctap")
                nc.vector.tensor_scalar_mul(ctap[:, :], xp[:, k:L + k],
                                            cw[:, k:k + 1])
                nc.vector.tensor_tensor(out=cacc[:, :], in0=cacc[:, :],
                                        in1=ctap[:, :], op=AL.add)
            xs = xsp.tile([128, L], bf16, tag="xs")
            nc.scalar.activation(xs[:, :], cacc[:, :], AF.Silu,
                                 bias=c128f[:, PF_CB + br:PF_CB + br + 1])
            dbc = mm_ps.tile([36, L], f32, tag="mm")
            xw = c128b[:, br * 36:(br + 1) * 36]
            nc.tensor.matmul(dbc[:, 0:512], lhsT=xw, rhs=xs[:, 0:512],
                             start=True, stop=True)
            nc.tensor.matmul(dbc[:, 512:768], lhsT=xw, rhs=xs[:, 512:768],
                             start=True, stop=True)
            dbc36 = dbcp.tile([36, L], bf16, tag="dbc36")
            nc.scalar.copy(out=dbc36[:, :], in_=dbc[:, :])
            # bounce B+C rows to DRAM for later partition-broadcast
            bdr = dramp.tile([32, L], bf16, tag="bdr")
            nc.sync.dma_start(out=bdr[:, :], in_=dbc36[4:36, :])
            xs_map[(b, br)] = xs
            dbc_map[(b, br)] = dbc36
            bc_dram[(b, br)] = bdr

        # ---- D2a sweep: dt logits (PE) + e^z (Exp set), all units ----
        dt_map = {}
        for b, br in units():
            dbc36 = dbc_map[(b, br)]
            dtps = mm_ps.tile([128, L], f32, tag="mm")
            dw = c4[:, br * 128:(br + 1) * 128]
            nc.tensor.matmul(dtps[:, 0:512], lhsT=dw, rhs=dbc36[0:4, 0:512],
                             start=True, stop=True)
            nc.tensor.matmul(dtps[:, 512:768], lhsT=dw, rhs=dbc36[0:4, 512:768],
                             start=True, stop=True)
            dtk = dtp.tile([128, L], bf16, tag="dt")
            nc.scalar.activation(dtk[:, :], dtps[:, :], AF.Exp,
                                 bias=c128f[:, PF_DTB + br:PF_DTB + br + 1])
            dt_map[(b, br)] = dtk
        # ---- D2b sweep: softplus ln(1 + e^z) in-place (Ln set), all units ----
        for b, br in units():
            dtk = dt_map[(b, br)]
            nc.scalar.activation(dtk[:, :], dtk[:, :], AF.Ln,
                                 bias=c128f[:, 75:76])
        # ---- D2c sweep: dA = exp(A_s*dt) (Exp set) + u = dt*xs, all units ----
        dA_map = {}
        u_map = {}
        for b, br in units():
            xs = xs_map[(b, br)]
            dtk = dt_map[(b, br)]
            Ac = c128f[:, PF_A + br * 16: PF_A + (br + 1) * 16]
            halves = []
            for hf in range(2):
                dA = dAp.tile([128, SH, TS], bf16, tag="dA")
                nc.vector.memset(dA[:, :, L:TS], 0.0)
                for sj in range(SH):
                    si = hf * SH + sj
                    nc.scalar.activation(dA[:, sj, 0:L], dtk[:, :], AF.Exp,
                                         scale=Ac[:, si:si + 1])
                halves.append(dA)
            dA_map[(b, br)] = halves
            u = up.tile([128, L], bf16, tag="u")
            nc.vector.tensor_tensor(out=u[:, :], in0=dtk[:, :], in1=xs[:, :],
                                    op=AL.mult)
            u_map[(b, br)] = u

        # ---- D3 sweep: broadcast, dBu, scan, y reduce, gate ----
        y_sum = {}
        for b in range(BL):
            yst = ysp.tile([128, L], bf16, tag="ysum", name=f"ysum{b}")
            y_sum[b] = yst

        tree_eng = nc.gpsimd if USE_GPSIMD_TREE else nc.vector

        for b, br in units():
            img_i = 0 if br < 2 else 1
            xs = xs_map[(b, br)]
            u = u_map[(b, br)]
            dA_a, dA_b = dA_map[(b, br)]
            bdr = bc_dram[(b, br)]
            uap = u[:, :]
            u_bc = bass.AP(tensor=uap.tensor, offset=uap.offset,
                           ap=[list(uap.ap[0]), [0, SH], list(uap.ap[-1])])
            # B/C broadcast: DRAM bounce rows flattened, partition-stride 0,
            # in 8-state halves (rows 0:8, 8:16 = B; 16:24, 24:32 = C)
            srcf = bdr[:, :].rearrange("a t -> (a t)")
            st0 = list(srcf.ap[-1])[0]

            def bcast_half(row0):
                t = bcp.tile([128, SH, L], bf16, tag="bc")
                sap = bass.AP(tensor=srcf.tensor, offset=srcf.offset + row0 * L,
                              ap=[[0, 128], [st0, SH * L]])
                nc.sync.dma_start(out=t[:, :, :], in_=sap)
                return t

            hs = []
            for hf, dA in ((0, dA_a), (1, dA_b)):
                bbc = bcast_half(hf * SH)
                dBu = dBup.tile([128, SH, TS], bf16, tag="dBu")
                nc.vector.memset(dBu[:, :, L:TS], 0.0)
                nc.vector.tensor_tensor(out=dBu[:, :, 0:L], in0=u_bc,
                                        in1=bbc[:, :, :], op=AL.mult)
                h = hp.tile([128, SH, TS], bf16, tag="h")
                nc.vector.tensor_tensor_scan(
                    out=h[:, :, :].rearrange("p s t -> p (s t)"),
                    data0=dA[:, :, :].rearrange("p s t -> p (s t)"),
                    data1=dBu[:, :, :].rearrange("p s t -> p (s t)"),
                    initial=0.0, op0=AL.mult, op1=AL.add)
                hs.append(h)
            # y = sum_s h*C: in-place mul per half + tree reduce
            for hf, h in ((0, hs[0]), (1, hs[1])):
                cbc = bcast_half(16 + hf * SH)
                nc.vector.tensor_tensor(out=h[:, :, 0:L], in0=h[:, :, 0:L],
                                        in1=cbc[:, :, :], op=AL.mult)
            h0 = hs[0]
            tree_eng.tensor_tensor(out=h0[:, :, 0:L], in0=h0[:, :, 0:L],
                                   in1=hs[1][:, :, 0:L], op=AL.add)
            tree_eng.tensor_tensor(out=h0[:, 0:4, 0:L], in0=h0[:, 0:4, 0:L],
                                   in1=h0[:, 4:8, 0:L], op=AL.add)
            nc.vector.tensor_tensor(out=h0[:, 0:2, 0:L], in0=h0[:, 0:2, 0:L],
                                    in1=h0[:, 2:4, 0:L], op=AL.add)
            yb = seqp.tile([128, L], bf16, tag="yb")
            nc.vector.tensor_tensor(out=yb[:, :], in0=h0[:, 0, 0:L],
                                    in1=h0[:, 1, 0:L], op=AL.add)
            # gate + accumulate (in-place on yb)
            nc.vector.scalar_tensor_tensor(
                out=yb[:, :], in0=xs[:, :],
                scalar=c128f[:, PF_D + br:PF_D + br + 1],
                in1=yb[:, :], op0=AL.mult, op1=AL.add)
            zt = sz[(img_i, b)]
            if br == 0:
                nc.vector.tensor_tensor(out=y_sum[b][:, :], in0=yb[:, :],
                                        in1=zt[:, :], op=AL.mult)
            else:
                zin = rev(zt[:, :]) if br == 1 else zt[:, :]
                nc.vector.tensor_tensor(out=yb[:, :], in0=yb[:, :], in1=zin,
                                        op=AL.mult)
                t2in = rev(yb[:, :]) if br == 1 else yb[:, :]
                nc.vector.tensor_tensor(out=y_sum[b][:, :], in0=y_sum[b][:, :],
                                        in1=t2in, op=AL.add)

        # ---- final head ----
        att_sb = outp.tile([1, BL * L], bf16, tag="big")
        vcol = vcol_b[:, :]
        for b in range(BL):
            lg = mm_ps.tile([1, L], f32, tag="mm")
            nc.tensor.matmul(lg[:, 0:512], lhsT=vcol, rhs=y_sum[b][:, 0:512],
                             start=True, stop=True)
            nc.tensor.matmul(lg[:, 512:768], lhsT=vcol, rhs=y_sum[b][:, 512:768],
                             start=True, stop=True)
            nc.scalar.activation(att_sb[:, b * L:(b + 1) * L], lg[:, :], AF.Sigmoid,
                                 scale=0.5, bias=cmisc[0:1, 0:1])
        nc.sync.dma_start(out=att_out, in_=att_sb[:, :])

    nc.compile()
    return nc


def _pack_params(inputs):
    import ml_dtypes
    gi = lambda k: np.asarray(inputs[k], dtype=np.float32)

    p128f = np.zeros((128, PF_NCOL), np.float32)
    p128b = np.zeros((128, 108), np.float32)
    tags = ("f", "b", "s")
    for t, tag in enumerate(tags):
        p128f[:, PF_A + t * 16: PF_A + 16 + t * 16] = -np.exp(gi("A_log_" + tag))
        p128f[:, PF_CW + t * 4: PF_CW + 4 + t * 4] = gi("conv_w_" + tag)
        p128f[:, PF_CB + t] = gi("conv_b_" + tag)
        p128f[:, PF_DTB + t] = gi("dtproj_b_" + tag)
        p128f[:, PF_D + t] = gi("D_" + tag)
        p128b[:, t * 36:(t + 1) * 36] = gi("xproj_w_" + tag).T
    p128f[:, PF_V] = gi("out_proj_w").T @ gi("post_w")[0]
    p128f[:, 74] = LN_EPS
    p128f[:, 75] = 1.0
    ln_g, ln_b = gi("ln_g"), gi("ln_b")
    w1t = gi("in_proj_w").T
    w2t = gi("in_proj_s_w").T
    b1 = ln_b @ w1t
    b2 = ln_b @ w2t
    p128f[:, PF_BE + 0] = b1[0:128]
    p128f[:, PF_BE + 1] = b1[128:256]
    p128f[:, PF_BE + 2] = b2[0:128]
    p128f[:, PF_BE + 3] = b2[128:256]

    p17f = np.zeros((17, 64), np.float32)
    p17f[0:16] = gi("pre_w").T / 64.0
    p17f[16] = gi("pre_b")

    p4b = np.zeros((4, 384), np.float32)
    for t, tag in enumerate(tags):
        p4b[:, t * 128:(t + 1) * 128] = gi("dtproj_w_" + tag).T

    p64b = np.zeros((64, 512), np.float32)
    p64b[:, 0:256] = w1t * ln_g[:, None]
    p64b[:, 256:512] = w2t * ln_g[:, None]

    miscf = np.zeros((1, 2), np.float32)
    miscf[0, 0] = 0.5 * float(gi("post_b").reshape(-1)[0])

    bf = ml_dtypes.bfloat16
    return {
        "p128f": p128f,
        "p128b": p128b.astype(bf),
        "p17f": p17f.astype(bf),
        "p4b": p4b.astype(bf),
        "p64b": p64b.astype(bf),
        "miscf": miscf,
    }


def get_nc():
    if "nc" not in _cached:
        _cached["nc"] = _build_nc()
    return _cached["nc"]


def make_in_maps(inputs):
    params = _pack_params(inputs)
    img1 = np.ascontiguousarray(np.asarray(inputs["img1_features"], np.float32))
    img2 = np.ascontiguousarray(np.asarray(inputs["img2_features"], np.float32))
    in_maps = []
    for c in range(NCORES):
        m = dict(params)
        m["img1"] = np.ascontiguousarray(img1[c * BL:(c + 1) * BL])
        m["img2"] = np.ascontiguousarray(img2[c * BL:(c + 1) * BL])
        in_maps.append(m)
    return in_maps


def kernel(**inputs):
    from concourse.bass_utils import run_bass_kernel_spmd

    nc = get_nc()
    in_maps = make_in_maps(inputs)
    res = run_bass_kernel_spmd(nc, in_maps, core_ids=list(range(NCORES)))
    outs = [np.asarray(r["att"], dtype=np.float32).reshape(BL, L)
            for r in res.results]
    att = np.concatenate(outs, axis=0) + 1e-6
    return att.reshape(B, C, 1, 1).astype(np.float32)


# revision 21
# speedup vs baseline: 1.0389x; 1.0389x over previous
"""CrossScanAttention (bimamba-v3) Trainium2 kernel.

Full inputs -> shard batch across 8 NeuronCores (2 batches/core) -> full output.
Self-contained: hardcodes all shapes; no sibling imports, no file reads.

Per-core pipeline (v2):
  pool4 (DVE multi-axis reduce) -> pre-proj+LN (PE + bn_stats, ln gamma/beta
  folded into in_proj weights) -> in_proj (PE, bias via ACT) ->
  D1 sweep (all 6 units): causal dwconv (DVE), silu (ACT, one table set),
    x_proj (PE), B/C rows bounced to DRAM;
  D2 sweep: dt_proj (PE), softplus (ACT exp+ln), dA = exp(A_s*dt) via 16 ACT
    exps (natural_log_exp set resident for the whole sweep), u = dt*xs (DVE);
  D3 sweep: B broadcast (DRAM bounce DMA), dBu (DVE), hardware
    tensor_tensor_scan in two 8-state halves, C broadcast, y = C-weighted
    reduce (tree levels 1-2 on GPSIMD, 3-4 on DVE), silu(z) gate, 3-branch sum;
  folded out_proj+post head (PE dot + ACT sigmoid).
"""

import numpy as np

# ---- problem constants ----
B, C, H, W = 16, 768, 32, 32
D_MODEL, D_INNER, D_STATE, DT_RANK, D_CONV = 64, 128, 16, 4, 4
LN_EPS = 1e-5
NCORES = 8
BL = B // NCORES          # 2 local batches per core
L = C                     # 768
TS = L + 2                # 770: segment with 2 zero break-cols (keeps 4B align)
SH = 8                    # states per scan half

USE_GPSIMD_TREE = False

_cached = {}

# p128f column layout
PF_A, PF_CW, PF_CB, PF_DTB, PF_D, PF_V, PF_BE = 0, 48, 60, 63, 66, 69, 70
PF_NCOL = 76


def _build_nc():
    import concourse.bass as bass
    import concourse.bacc as bacc
    import concourse.tile as tile
    import concourse.mybir as mybir
    from concourse.masks import make_identity
    from contextlib import ExitStack

    f32 = mybir.dt.float32
    bf16 = mybir.dt.bfloat16
    AL = mybir.AluOpType
    AF = mybir.ActivationFunctionType
    AX = mybir.AxisListType

    nc = bacc.Bacc("TRN2", target_bir_lowering=False, debug=False)

    img1 = nc.dram_tensor("img1", (BL, C, H, W), f32, kind="ExternalInput").ap()
    img2 = nc.dram_tensor("img2", (BL, C, H, W), f32, kind="ExternalInput").ap()
    p128f = nc.dram_tensor("p128f", (128, PF_NCOL), f32, kind="ExternalInput").ap()
    p128b = nc.dram_tensor("p128b", (128, 108), bf16, kind="ExternalInput").ap()
    p17f = nc.dram_tensor("p17f", (17, 64), bf16, kind="ExternalInput").ap()
    p4b = nc.dram_tensor("p4b", (4, 384), bf16, kind="ExternalInput").ap()
    p64b = nc.dram_tensor("p64b", (64, 512), bf16, kind="ExternalInput").ap()
    miscf = nc.dram_tensor("miscf", (1, 2), f32, kind="ExternalInput").ap()
    att_out = nc.dram_tensor("att", (1, BL * L), bf16, kind="ExternalOutput").ap()

    def rev(ap):
        """View with the last free dim reversed."""
        steps = [list(x) for x in ap.ap]
        st, n = steps[-1]
        newap = steps[:-1] + [[-st, n]]
        return bass.AP(tensor=ap.tensor, offset=ap.offset + st * (n - 1), ap=newap)

    def units():
        # match stage C xpad creation order: (b, br01) for i=0, then br2
        for b in range(BL):
            for br in (0, 1):
                yield b, br
        for b in range(BL):
            yield b, 2

    with nc.allow_low_precision("bf16 intermediate precision is sufficient"), \
         tile.TileContext(nc) as tc, ExitStack() as ctx:
        consts = ctx.enter_context(tc.tile_pool(name="consts", bufs=1))
        imgp = ctx.enter_context(tc.tile_pool(name="imgp", bufs=3))
        small_ps = ctx.enter_context(tc.tile_pool(name="small_ps", bufs=4, space="PSUM"))
        mm_ps = ctx.enter_context(tc.tile_pool(name="mm_ps", bufs=2, space="PSUM"))
        stats = ctx.enter_context(tc.tile_pool(name="stats", bufs=4))
        xtp = ctx.enter_context(tc.tile_pool(name="xtp", bufs=3))
        szp = ctx.enter_context(tc.tile_pool(name="szp", bufs=4))
        xpadp = ctx.enter_context(tc.tile_pool(name="xpadp", bufs=3))
        ysp = ctx.enter_context(tc.tile_pool(name="ysp", bufs=2))
        dbcp = ctx.enter_context(tc.tile_pool(name="dbcp", bufs=6))
        xsp = ctx.enter_context(tc.tile_pool(name="xsp", bufs=6))
        up = ctx.enter_context(tc.tile_pool(name="up", bufs=3))
        seqp = ctx.enter_context(tc.tile_pool(name="seqp", bufs=2))
        dtp = ctx.enter_context(tc.tile_pool(name="dtp", bufs=4))
        dAp = ctx.enter_context(tc.tile_pool(name="dAp", bufs=3))
        hp = ctx.enter_context(tc.tile_pool(name="hp", bufs=3))
        bcp = ctx.enter_context(tc.tile_pool(name="bcp", bufs=3))
        dramp = ctx.enter_context(tc.tile_pool(name="dramp", bufs=6, space="DRAM"))
        outp = ctx.enter_context(tc.tile_pool(name="outp", bufs=1))

        # ---- constants ----
        c128f = consts.tile([128, PF_NCOL], f32)
        nc.sync.dma_start(out=c128f, in_=p128f)
        c128b = consts.tile([128, 108], bf16)
        nc.sync.dma_start(out=c128b, in_=p128b)
        c17 = consts.tile([16, 64], bf16)
        nc.sync.dma_start(out=c17, in_=p17f[0:16, :])
        c_preb = consts.tile([1, 64], bf16)
        nc.sync.dma_start(out=c_preb, in_=p17f[16:17, :])
        ones1 = consts.tile([1, 128], bf16)
        nc.vector.memset(ones1[:, :], 1.0)
        c4 = consts.tile([4, 384], bf16)
        nc.sync.dma_start(out=c4, in_=p4b)
        c64 = consts.tile([64, 512], bf16)
        nc.sync.dma_start(out=c64, in_=p64b)
        cmisc = consts.tile([1, 2], f32)
        nc.sync.dma_start(out=cmisc, in_=miscf)
        identb = consts.tile([128, 128], bf16)
        make_identity(nc, identb[:, :])
        vcol_b = consts.tile([128, 1], bf16)
        nc.scalar.copy(out=vcol_b[:, :], in_=c128f[:, PF_V:PF_V + 1])
        zeros32 = consts.tile([32, 2], bf16)
        nc.vector.memset(zeros32[:, :], 0.0)

        # ---- stage A: pool -> pooledT [16, 4*768] bf16 ----
        pooledT = outp.tile([16, 4 * L], bf16, tag="big")
        imgs = [img1, img2]
        for i in range(2):
            for b in range(BL):
                for k in range(6):
                    it = imgp.tile([128, 1024], f32, tag="imgtile")
                    src = imgs[i][b, k * 128:(k + 1) * 128, :, :].rearrange(
                        "c h w -> c (h w)")
                    nc.sync.dma_start(out=it, in_=src)
                    v5 = it[:, :].rearrange("p (jr hb jc wb) -> p jr jc hb wb",
                                            jr=4, hb=8, jc=4, wb=8)
                    psum = stats.tile([128, 16], bf16, tag="poolsum")
                    nc.vector.tensor_reduce(out=psum, in_=v5, axis=AX.XY, op=AL.add)
                    pt_ps = small_ps.tile([16, 128], bf16, tag="sm")
                    nc.tensor.transpose(pt_ps[:, :], psum[:, :], identb[:, :])
                    col = (i * BL + b) * L + k * 128
                    nc.scalar.copy(out=pooledT[0:16, col:col + 128], in_=pt_ps)

        # ---- stage B: pre-proj + LN -> xT[(i,b)] [64, 768] bf16 ----
        xT = {}
        for i in range(2):
            for b in range(BL):
                xt = xtp.tile([64, L], bf16, tag="xT")
                for k in range(6):
                    col = (i * BL + b) * L + k * 128
                    xp_ps = small_ps.tile([128, 64], f32, tag="sm")
                    nc.tensor.matmul(xp_ps[:, :], lhsT=pooledT[:, col:col + 128],
                                     rhs=c17[:, :], start=True, stop=False)
                    nc.tensor.matmul(xp_ps[:, :], lhsT=ones1[:, :],
                                     rhs=c_preb[:, :], start=False, stop=True)
                    st6 = stats.tile([128, 6], f32, tag="bnst")
                    nc.vector.bn_stats(out=st6, in_=xp_ps)
                    mv = stats.tile([128, 2], f32, tag="bnmv")
                    nc.vector.bn_aggr(out=mv, in_=st6)
                    sq = stats.tile([128, 1], f32, tag="sq")
                    nc.scalar.activation(sq[:, :], mv[:, 1:2], AF.Sqrt,
                                         bias=c128f[:, 74:75])
                    rs = stats.tile([128, 1], f32, tag="rs")
                    nc.vector.reciprocal(out=rs[:, :], in_=sq[:, :])
                    xn = stats.tile([128, 64], bf16, tag="xn")
                    nc.vector.tensor_scalar(out=xn[:, :], in0=xp_ps[:, :],
                                            scalar1=mv[:, 0:1], scalar2=rs[:, 0:1],
                                            op0=AL.subtract, op1=AL.mult)
                    xn_ps = small_ps.tile([64, 128], bf16, tag="sm")
                    nc.tensor.transpose(xn_ps[:, :], xn[:, :], identb[:, :])
                    nc.vector.tensor_copy(out=xt[:, k * 128:(k + 1) * 128],
                                          in_=xn_ps[:, :])
                xT[(i, b)] = xt

        # ---- stage C: in_proj -> xpad[(b,br)] [128, 771] bf16, sz[(i,b)] ----
        xpad = {}
        sz = {}
        for i in range(2):
            for b in range(BL):
                for h in range(2):
                    ps = mm_ps.tile([128, L], f32, tag="mm")
                    wsl = c64[:, i * 256 + h * 128: i * 256 + (h + 1) * 128]
                    nc.tensor.matmul(ps[:, 0:512], lhsT=wsl, rhs=xT[(i, b)][:, 0:512],
                                     start=True, stop=True)
                    nc.tensor.matmul(ps[:, 512:768], lhsT=wsl,
                                     rhs=xT[(i, b)][:, 512:768], start=True, stop=True)
                    be = c128f[:, PF_BE + i * 2 + h: PF_BE + i * 2 + h + 1]
                    if h == 0:
                        brs = [0, 1] if i == 0 else [2]
                        for br in brs:
                            xp = xpadp.tile([128, L + 3], bf16, tag="xpad")
                            nc.vector.memset(xp[:, 0:3], 0.0)
                            src = ps[:, :] if br != 1 else rev(ps[:, :])
                            nc.vector.tensor_scalar_add(xp[:, 3:L + 3], src, be)
                            xpad[(b, br)] = xp
                    else:
                        z = szp.tile([128, L], bf16, tag="sz")
                        nc.scalar.activation(z[:, :], ps[:, :], AF.Silu, bias=be)
                        sz[(i, b)] = z

        # ---- D1 sweep: conv + silu + xproj + B/C bounce (Silu set resident) ----
        xs_map = {}
        dbc_map = {}
        bc_dram = {}
        for b, br in units():
            xp = xpad[(b, br)]
            cw = c128f[:, PF_CW + br * 4: PF_CW + (br + 1) * 4]
            cacc = seqp.tile([128, L], bf16, tag="cacc")
            nc.vector.tensor_scalar_mul(cacc[:, :], xp[:, 0:L], cw[:, 0:1])
            for k in range(1, 4):
                ctap = seqp.tile([128, L], bf16, tag="ctap")
                nc.vector.tensor_scalar_mul(ctap[:, :], xp[:, k:L + k],
                                            cw[:, k:k + 1])
                nc.vector.tensor_tensor(out=cacc[:, :], in0=cacc[:, :],
                                        in1=ctap[:, :], op=AL.add)
            xs = xsp.tile([128, L], bf16, tag="xs")
            nc.scalar.activation(xs[:, :], cacc[:, :], AF.Silu,
                                 bias=c128f[:, PF_CB + br:PF_CB + br + 1])
            dbc = mm_ps.tile([36, L], f32, tag="mm")
            xw = c128b[:, br * 36:(br + 1) * 36]
            nc.tensor.matmul(dbc[:, 0:512], lhsT=xw, rhs=xs[:, 0:512],
                             start=True, stop=True)
            nc.tensor.matmul(dbc[:, 512:768], lhsT=xw, rhs=xs[:, 512:768],
                             start=True, stop=True)
            dbc36 = dbcp.tile([36, L], bf16, tag="dbc36")
            nc.scalar.copy(out=dbc36[:, :], in_=dbc[:, :])
            # bounce B+C rows to DRAM for later partition-broadcast
            # (TS layout: 2 zero break cols per row for direct scan consumption)
            bdr = dramp.tile([32, TS], bf16, tag="bdr")
            nc.sync.dma_start(out=bdr[:, 0:L], in_=dbc36[4:36, :])
            nc.sync.dma_start(out=bdr[:, L:TS], in_=zeros32[:, :])
            xs_map[(b, br)] = xs
            dbc_map[(b, br)] = dbc36
            bc_dram[(b, br)] = bdr

        # ---- D2a sweep: dt logits (PE) + e^z (Exp set), all units ----
        dt_map = {}
        for b, br in units():
            dbc36 = dbc_map[(b, br)]
            dtps = mm_ps.tile([128, L], f32, tag="mm")
            dw = c4[:, br * 128:(br + 1) * 128]
            nc.tensor.matmul(dtps[:, 0:512], lhsT=dw, rhs=dbc36[0:4, 0:512],
                             start=True, stop=True)
            nc.tensor.matmul(dtps[:, 512:768], lhsT=dw, rhs=dbc36[0:4, 512:768],
                             start=True, stop=True)
            dtk = dtp.tile([128, L], bf16, tag="dt")
            nc.scalar.activation(dtk[:, :], dtps[:, :], AF.Exp,
                                 bias=c128f[:, PF_DTB + br:PF_DTB + br + 1])
            dt_map[(b, br)] = dtk
        # ---- D2b sweep: softplus ln(1 + e^z) in-place (Ln set), all units ----
        for b, br in units():
            dtk = dt_map[(b, br)]
            nc.scalar.activation(dtk[:, :], dtk[:, :], AF.Ln,
                                 bias=c128f[:, 75:76])
        # ---- D2c sweep: dA = exp(A_s*dt) (Exp set) + u = dt*xs, all units ----
        dA_map = {}
        u_map = {}
        for b, br in units():
            xs = xs_map[(b, br)]
            dtk = dt_map[(b, br)]
            Ac = c128f[:, PF_A + br * 16: PF_A + (br + 1) * 16]
            halves = []
            for hf in range(2):
                dA = dAp.tile([128, SH, TS], bf16, tag="dA")
                nc.vector.memset(dA[:, :, L:TS], 0.0)
                for sj in range(SH):
                    si = hf * SH + sj
                    nc.scalar.activation(dA[:, sj, 0:L], dtk[:, :], AF.Exp,
                                         scale=Ac[:, si:si + 1])
                halves.append(dA)
            dA_map[(b, br)] = halves
            u = up.tile([128, L], bf16, tag="u")
            nc.vector.tensor_tensor(out=u[:, :], in0=dtk[:, :], in1=xs[:, :],
                                    op=AL.mult)
            u_map[(b, br)] = u

        # ---- D3 sweep: broadcast, dBu, scan, y reduce, gate ----
        y_sum = {}
        for b in range(BL):
            yst = ysp.tile([128, L], bf16, tag="ysum", name=f"ysum{b}")
            y_sum[b] = yst

        tree_eng = nc.gpsimd if USE_GPSIMD_TREE else nc.vector

        for b, br in units():
            img_i = 0 if br < 2 else 1
            xs = xs_map[(b, br)]
            u = u_map[(b, br)]
            dA_a, dA_b = dA_map[(b, br)]
            bdr = bc_dram[(b, br)]
            uap = u[:, :]
            u_bc = bass.AP(tensor=uap.tensor, offset=uap.offset,
                           ap=[list(uap.ap[0]), [0, SH], list(uap.ap[-1])])
            # B/C broadcast: DRAM bounce rows (TS-wide, breaks included)
            # flattened, partition-stride 0, in 8-state halves
            srcf = bdr[:, :].rearrange("a t -> (a t)")
            st0 = list(srcf.ap[-1])[0]

            def bcast_half(row0):
                t = bcp.tile([128, SH, TS], bf16, tag="bc")
                sap = bass.AP(tensor=srcf.tensor, offset=srcf.offset + row0 * TS,
                              ap=[[0, 128], [st0, SH * TS]])
                nc.sync.dma_start(out=t[:, :, :], in_=sap)
                return t

            hs = []
            for hf, dA in ((0, dA_a), (1, dA_b)):
                bbc = bcast_half(hf * SH)
                # dBu = u * B in place on the broadcast tile (breaks stay 0)
                nc.vector.tensor_tensor(out=bbc[:, :, 0:L], in0=u_bc,
                                        in1=bbc[:, :, 0:L], op=AL.mult)
                h = hp.tile([128, SH, TS], bf16, tag="h")
                nc.vector.tensor_tensor_scan(
                    out=h[:, :, :].rearrange("p s t -> p (s t)"),
                    data0=dA[:, :, :].rearrange("p s t -> p (s t)"),
                    data1=bbc[:, :, :].rearrange("p s t -> p (s t)"),
                    initial=0.0, op0=AL.mult, op1=AL.add)
                hs.append(h)
            # y = sum_s h*C: in-place mul per half + tree reduce
            for hf, h in ((0, hs[0]), (1, hs[1])):
                cbc = bcast_half(16 + hf * SH)
                nc.vector.tensor_tensor(out=h[:, :, 0:L], in0=h[:, :, 0:L],
                                        in1=cbc[:, :, 0:L], op=AL.mult)
            h0 = hs[0]
            tree_eng.tensor_tensor(out=h0[:, :, 0:L], in0=h0[:, :, 0:L],
                                   in1=hs[1][:, :, 0:L], op=AL.add)
            tree_eng.tensor_tensor(out=h0[:, 0:4, 0:L], in0=h0[:, 0:4, 0:L],
                                   in1=h0[:, 4:8, 0:L], op=AL.add)
            nc.vector.tensor_tensor(out=h0[:, 0:2, 0:L], in0=h0[:, 0:2, 0:L],
                                    in1=h0[:, 2:4, 0:L], op=AL.add)
            yb = seqp.tile([128, L], bf16, tag="yb")
            nc.vector.tensor_tensor(out=yb[:, :], in0=h0[:, 0, 0:L],
                                    in1=h0[:, 1, 0:L], op=AL.add)
            # gate + accumulate (in-place on yb)
            nc.vector.scalar_tensor_tensor(
                out=yb[:, :], in0=xs[:, :],
                scalar=c128f[:, PF_D + br:PF_D + br + 1],
                in1=yb[:, :], op0=AL.mult, op1=AL.add)
            zt = sz[(img_i, b)]
            if br == 0:
                nc.vector.tensor_tensor(out=y_sum[b][:, :], in0=yb[:, :],
                                        in1=zt[:, :], op=AL.mult)
            else:
                zin = rev(zt[:, :]) if br == 1 else zt[:, :]
                nc.vector.tensor_tensor(out=yb[:, :], in0=yb[:, :], in1=zin,
                                        op=AL.mult)
                t2in = rev(yb[:, :]) if br == 1 else yb[:, :]
                nc.vector.tensor_tensor(out=y_sum[b][:, :], in0=y_sum[b][:, :],
                                        in1=t2in, op=AL.add)

        # ---- final head ----
        att_sb = outp.tile([1, BL * L], bf16, tag="big")
        vcol = vcol_b[:, :]
        for b in range(BL):
            lg = mm_ps.tile([1, L], f32, tag="mm")
            nc.tensor.matmul(lg[:, 0:512], lhsT=vcol, rhs=y_sum[b][:, 0:512],
                             start=True, stop=True)
            nc.tensor.matmul(lg[:, 512:768], lhsT=vcol, rhs=y_sum[b][:, 512:768],
                             start=True, stop=True)
            nc.scalar.activation(att_sb[:, b * L:(b + 1) * L], lg[:, :], AF.Sigmoid,
                                 scale=0.5, bias=cmisc[0:1, 0:1])
        nc.sync.dma_start(out=att_out, in_=att_sb[:, :])

    nc.compile()
    return nc


def _pack_params(inputs):
    import ml_dtypes
    gi = lambda k: np.asarray(inputs[k], dtype=np.float32)

    p128f = np.zeros((128, PF_NCOL), np.float32)
    p128b = np.zeros((128, 108), np.float32)
    tags = ("f", "b", "s")
    for t, tag in enumerate(tags):
        p128f[:, PF_A + t * 16: PF_A + 16 + t * 16] = -np.exp(gi("A_log_" + tag))
        p128f[:, PF_CW + t * 4: PF_CW + 4 + t * 4] = gi("conv_w_" + tag)
        p128f[:, PF_CB + t] = gi("conv_b_" + tag)
        p128f[:, PF_DTB + t] = gi("dtproj_b_" + tag)
        p128f[:, PF_D + t] = gi("D_" + tag)
        p128b[:, t * 36:(t + 1) * 36] = gi("xproj_w_" + tag).T
    p128f[:, PF_V] = gi("out_proj_w").T @ gi("post_w")[0]
    p128f[:, 74] = LN_EPS
    p128f[:, 75] = 1.0
    ln_g, ln_b = gi("ln_g"), gi("ln_b")
    w1t = gi("in_proj_w").T
    w2t = gi("in_proj_s_w").T
    b1 = ln_b @ w1t
    b2 = ln_b @ w2t
    p128f[:, PF_BE + 0] = b1[0:128]
    p128f[:, PF_BE + 1] = b1[128:256]
    p128f[:, PF_BE + 2] = b2[0:128]
    p128f[:, PF_BE + 3] = b2[128:256]

    p17f = np.zeros((17, 64), np.float32)
    p17f[0:16] = gi("pre_w").T / 64.0
    p17f[16] = gi("pre_b")

    p4b = np.zeros((4, 384), np.float32)
    for t, tag in enumerate(tags):
        p4b[:, t * 128:(t + 1) * 128] = gi("dtproj_w_" + tag).T

    p64b = np.zeros((64, 512), np.float32)
    p64b[:, 0:256] = w1t * ln_g[:, None]
    p64b[:, 256:512] = w2t * ln_g[:, None]

    miscf = np.zeros((1, 2), np.float32)
    miscf[0, 0] = 0.5 * float(gi("post_b").reshape(-1)[0])

    bf = ml_dtypes.bfloat16
    return {
        "p128f": p128f,
        "p128b": p128b.astype(bf),
        "p17f": p17f.astype(bf),
        "p4b": p4b.astype(bf),
        "p64b": p64b.astype(bf),
        "miscf": miscf,
    }


def get_nc():
    if "nc" not in _cached:
        _cached["nc"] = _build_nc()
    return _cached["nc"]


def make_in_maps(inputs):
    params = _pack_params(inputs)
    img1 = np.ascontiguousarray(np.asarray(inputs["img1_features"], np.float32))
    img2 = np.ascontiguousarray(np.asarray(inputs["img2_features"], np.float32))
    in_maps = []
    for c in range(NCORES):
        m = dict(params)
        m["img1"] = np.ascontiguousarray(img1[c * BL:(c + 1) * BL])
        m["img2"] = np.ascontiguousarray(img2[c * BL:(c + 1) * BL])
        in_maps.append(m)
    return in_maps


def kernel(**inputs):
    from concourse.bass_utils import run_bass_kernel_spmd

    nc = get_nc()
    in_maps = make_in_maps(inputs)
    res = run_bass_kernel_spmd(nc, in_maps, core_ids=list(range(NCORES)))
    outs = [np.asarray(r["att"], dtype=np.float32).reshape(BL, L)
            for r in res.results]
    att = np.concatenate(outs, axis=0) + 1e-6
    return att.reshape(B, C, 1, 1).astype(np.float32)


# revision 24
# speedup vs baseline: 1.0421x; 1.0030x over previous
"""CrossScanAttention (bimamba-v3) Trainium2 kernel.

Full inputs -> shard batch across 8 NeuronCores (2 batches/core) -> full output.
Self-contained: hardcodes all shapes; no sibling imports, no file reads.

Per-core pipeline (v2):
  pool4 (DVE multi-axis reduce) -> pre-proj+LN (PE + bn_stats, ln gamma/beta
  folded into in_proj weights) -> in_proj (PE, bias via ACT) ->
  D1 sweep (all 6 units): causal dwconv (DVE), silu (ACT, one table set),
    x_proj (PE), B/C rows bounced to DRAM;
  D2 sweep: dt_proj (PE), softplus (ACT exp+ln), dA = exp(A_s*dt) via 16 ACT
    exps (natural_log_exp set resident for the whole sweep), u = dt*xs (DVE);
  D3 sweep: B broadcast (DRAM bounce DMA), dBu (DVE), hardware
    tensor_tensor_scan in two 8-state halves, C broadcast, y = C-weighted
    reduce (tree levels 1-2 on GPSIMD, 3-4 on DVE), silu(z) gate, 3-branch sum;
  folded out_proj+post head (PE dot + ACT sigmoid).
"""

import numpy as np

# ---- problem constants ----
B, C, H, W = 16, 768, 32, 32
D_MODEL, D_INNER, D_STATE, DT_RANK, D_CONV = 64, 128, 16, 4, 4
LN_EPS = 1e-5
NCORES = 8
BL = B // NCORES          # 2 local batches per core
L = C                     # 768
TS = L + 2                # 770: segment with 2 zero break-cols (keeps 4B align)
SH = 8                    # states per scan half

USE_GPSIMD_TREE = False

_cached = {}

# p128f column layout
PF_A, PF_CW, PF_CB, PF_DTB, PF_D, PF_V, PF_BE = 0, 48, 60, 63, 66, 69, 70
PF_NCOL = 76


def _build_nc():
    import concourse.bass as bass
    import concourse.bacc as bacc
    import concourse.tile as tile
    import concourse.mybir as mybir
    from concourse.masks import make_identity
    from contextlib import ExitStack

    f32 = mybir.dt.float32
    bf16 = mybir.dt.bfloat16
    AL = mybir.AluOpType
    AF = mybir.ActivationFunctionType
    AX = mybir.AxisListType

    nc = bacc.Bacc("TRN2", target_bir_lowering=False, debug=False)

    img1 = nc.dram_tensor("img1", (BL, C, H, W), f32, kind="ExternalInput").ap()
    img2 = nc.dram_tensor("img2", (BL, C, H, W), f32, kind="ExternalInput").ap()
    p128f = nc.dram_tensor("p128f", (128, PF_NCOL), f32, kind="ExternalInput").ap()
    p128b = nc.dram_tensor("p128b", (128, 108), bf16, kind="ExternalInput").ap()
    p17f = nc.dram_tensor("p17f", (17, 64), bf16, kind="ExternalInput").ap()
    p4b = nc.dram_tensor("p4b", (4, 384), bf16, kind="ExternalInput").ap()
    p64b = nc.dram_tensor("p64b", (64, 512), bf16, kind="ExternalInput").ap()
    miscf = nc.dram_tensor("miscf", (1, 2), f32, kind="ExternalInput").ap()
    att_out = nc.dram_tensor("att", (1, BL * L), bf16, kind="ExternalOutput").ap()

    def rev(ap):
        """View with the last free dim reversed."""
        steps = [list(x) for x in ap.ap]
        st, n = steps[-1]
        newap = steps[:-1] + [[-st, n]]
        return bass.AP(tensor=ap.tensor, offset=ap.offset + st * (n - 1), ap=newap)

    def units():
        # match stage C xpad creation order: (b, br01) for i=0, then br2
        for b in range(BL):
            for br in (0, 1):
                yield b, br
        for b in range(BL):
            yield b, 2

    with nc.allow_low_precision("bf16 intermediate precision is sufficient"), \
         tile.TileContext(nc) as tc, ExitStack() as ctx:
        consts = ctx.enter_context(tc.tile_pool(name="consts", bufs=1))
        imgp = ctx.enter_context(tc.tile_pool(name="imgp", bufs=3))
        small_ps = ctx.enter_context(tc.tile_pool(name="small_ps", bufs=4, space="PSUM"))
        mm_ps = ctx.enter_context(tc.tile_pool(name="mm_ps", bufs=2, space="PSUM"))
        stats = ctx.enter_context(tc.tile_pool(name="stats", bufs=4))
        xtp = ctx.enter_context(tc.tile_pool(name="xtp", bufs=3))
        szp = ctx.enter_context(tc.tile_pool(name="szp", bufs=4))
        xpadp = ctx.enter_context(tc.tile_pool(name="xpadp", bufs=3))
        ysp = ctx.enter_context(tc.tile_pool(name="ysp", bufs=2))
        dbcp = ctx.enter_context(tc.tile_pool(name="dbcp", bufs=6))
        xsp = ctx.enter_context(tc.tile_pool(name="xsp", bufs=6))
        up = ctx.enter_context(tc.tile_pool(name="up", bufs=3))
        seqp = ctx.enter_context(tc.tile_pool(name="seqp", bufs=2))
        dtp = ctx.enter_context(tc.tile_pool(name="dtp", bufs=4))
        dAp = ctx.enter_context(tc.tile_pool(name="dAp", bufs=3))
        hp = ctx.enter_context(tc.tile_pool(name="hp", bufs=3))
        bcp = ctx.enter_context(tc.tile_pool(name="bcp", bufs=3))
        dramp = ctx.enter_context(tc.tile_pool(name="dramp", bufs=6, space="DRAM"))
        outp = ctx.enter_context(tc.tile_pool(name="outp", bufs=1))

        # ---- constants ----
        c128f = consts.tile([128, PF_NCOL], f32)
        nc.sync.dma_start(out=c128f, in_=p128f)
        c128b = consts.tile([128, 108], bf16)
        nc.sync.dma_start(out=c128b, in_=p128b)
        c17 = consts.tile([16, 64], bf16)
        nc.sync.dma_start(out=c17, in_=p17f[0:16, :])
        c_preb = consts.tile([1, 64], bf16)
        nc.sync.dma_start(out=c_preb, in_=p17f[16:17, :])
        ones1 = consts.tile([1, 128], bf16)
        nc.vector.memset(ones1[:, :], 1.0)
        c4 = consts.tile([4, 384], bf16)
        nc.sync.dma_start(out=c4, in_=p4b)
        c64 = consts.tile([64, 512], bf16)
        nc.sync.dma_start(out=c64, in_=p64b)
        cmisc = consts.tile([1, 2], f32)
        nc.sync.dma_start(out=cmisc, in_=miscf)
        identb = consts.tile([128, 128], bf16)
        make_identity(nc, identb[:, :])
        vcol_b = consts.tile([128, 1], bf16)
        nc.scalar.copy(out=vcol_b[:, :], in_=c128f[:, PF_V:PF_V + 1])
        zeros32 = consts.tile([32, 2], bf16)
        nc.vector.memset(zeros32[:, :], 0.0)

        # ---- stage A: pool -> pooledT [16, 4*768] bf16 ----
        pooledT = outp.tile([16, 4 * L], bf16, tag="big")
        imgs = [img1, img2]
        for i in range(2):
            for b in range(BL):
                for k in range(6):
                    it = imgp.tile([128, 1024], f32, tag="imgtile")
                    src = imgs[i][b, k * 128:(k + 1) * 128, :, :].rearrange(
                        "c h w -> c (h w)")
                    nc.sync.dma_start(out=it, in_=src)
                    v5 = it[:, :].rearrange("p (jr hb jc wb) -> p jr jc hb wb",
                                            jr=4, hb=8, jc=4, wb=8)
                    psum = stats.tile([128, 16], bf16, tag="poolsum")
                    nc.vector.tensor_reduce(out=psum, in_=v5, axis=AX.XY, op=AL.add)
                    pt_ps = small_ps.tile([16, 128], bf16, tag="sm")
                    nc.tensor.transpose(pt_ps[:, :], psum[:, :], identb[:, :])
                    col = (i * BL + b) * L + k * 128
                    nc.scalar.copy(out=pooledT[0:16, col:col + 128], in_=pt_ps)

        # ---- stage B: pre-proj + LN -> xT[(i,b)] [64, 768] bf16 ----
        xT = {}
        for i in range(2):
            for b in range(BL):
                xt = xtp.tile([64, L], bf16, tag="xT")
                for k in range(6):
                    col = (i * BL + b) * L + k * 128
                    xp_ps = small_ps.tile([128, 64], f32, tag="sm")
                    nc.tensor.matmul(xp_ps[:, :], lhsT=pooledT[:, col:col + 128],
                                     rhs=c17[:, :], start=True, stop=False)
                    nc.tensor.matmul(xp_ps[:, :], lhsT=ones1[:, :],
                                     rhs=c_preb[:, :], start=False, stop=True)
                    st6 = stats.tile([128, 6], f32, tag="bnst")
                    nc.vector.bn_stats(out=st6, in_=xp_ps)
                    mv = stats.tile([128, 2], f32, tag="bnmv")
                    nc.vector.bn_aggr(out=mv, in_=st6)
                    sq = stats.tile([128, 1], f32, tag="sq")
                    nc.scalar.activation(sq[:, :], mv[:, 1:2], AF.Sqrt,
                                         bias=c128f[:, 74:75])
                    rs = stats.tile([128, 1], f32, tag="rs")
                    nc.vector.reciprocal(out=rs[:, :], in_=sq[:, :])
                    xn = stats.tile([128, 64], bf16, tag="xn")
                    nc.vector.tensor_scalar(out=xn[:, :], in0=xp_ps[:, :],
                                            scalar1=mv[:, 0:1], scalar2=rs[:, 0:1],
                                            op0=AL.subtract, op1=AL.mult)
                    xn_ps = small_ps.tile([64, 128], bf16, tag="sm")
                    nc.tensor.transpose(xn_ps[:, :], xn[:, :], identb[:, :])
                    nc.vector.tensor_copy(out=xt[:, k * 128:(k + 1) * 128],
                                          in_=xn_ps[:, :])
                xT[(i, b)] = xt

        # ---- stage C: in_proj -> xpad[(b,br)] [128, 771] bf16, sz[(i,b)] ----
        xpad = {}
        sz = {}
        for i in range(2):
            for b in range(BL):
                for h in range(2):
                    ps = mm_ps.tile([128, L], f32, tag="mm")
                    wsl = c64[:, i * 256 + h * 128: i * 256 + (h + 1) * 128]
                    nc.tensor.matmul(ps[:, 0:512], lhsT=wsl, rhs=xT[(i, b)][:, 0:512],
                                     start=True, stop=True)
                    nc.tensor.matmul(ps[:, 512:768], lhsT=wsl,
                                     rhs=xT[(i, b)][:, 512:768], start=True, stop=True)
                    be = c128f[:, PF_BE + i * 2 + h: PF_BE + i * 2 + h + 1]
                    if h == 0:
                        brs = [0, 1] if i == 0 else [2]
                        for br in brs:
                            xp = xpadp.tile([128, L + 3], bf16, tag="xpad")
                            nc.vector.memset(xp[:, 0:3], 0.0)
                            src = ps[:, :] if br != 1 else rev(ps[:, :])
                            nc.vector.tensor_scalar_add(xp[:, 3:L + 3], src, be)
                            xpad[(b, br)] = xp
                    else:
                        z = szp.tile([128, L], bf16, tag="sz")
                        nc.scalar.activation(z[:, :], ps[:, :], AF.Silu, bias=be)
                        sz[(i, b)] = z

        # ---- D1 sweep: conv + silu + xproj + B/C bounce (Silu set resident) ----
        xs_map = {}
        dbc_map = {}
        bc_dram = {}
        for b, br in units():
            xp = xpad[(b, br)]
            cw = c128f[:, PF_CW + br * 4: PF_CW + (br + 1) * 4]
            cacc = seqp.tile([128, L], bf16, tag="cacc")
            nc.vector.tensor_scalar_mul(cacc[:, :], xp[:, 0:L], cw[:, 0:1])
            for k in range(1, 4):
                ctap = seqp.tile([128, L], bf16, tag="ctap")
                nc.vector.tensor_scalar_mul(ctap[:, :], xp[:, k:L + k],
                                            cw[:, k:k + 1])
                nc.vector.tensor_tensor(out=cacc[:, :], in0=cacc[:, :],
                                        in1=ctap[:, :], op=AL.add)
            xs = xsp.tile([128, L], bf16, tag="xs")
            nc.scalar.activation(xs[:, :], cacc[:, :], AF.Silu,
                                 bias=c128f[:, PF_CB + br:PF_CB + br + 1])
            dbc = mm_ps.tile([36, L], f32, tag="mm")
            xw = c128b[:, br * 36:(br + 1) * 36]
            nc.tensor.matmul(dbc[:, 0:512], lhsT=xw, rhs=xs[:, 0:512],
                             start=True, stop=True)
            nc.tensor.matmul(dbc[:, 512:768], lhsT=xw, rhs=xs[:, 512:768],
                             start=True, stop=True)
            dbc36 = dbcp.tile([36, L], bf16, tag="dbc36")
            nc.scalar.copy(out=dbc36[:, :], in_=dbc[:, :])
            # bounce B+C rows to DRAM for later partition-broadcast
            # (TS layout: 2 zero break cols per row for direct scan consumption)
            bdr = dramp.tile([32, TS], bf16, tag="bdr")
            nc.sync.dma_start(out=bdr[:, 0:L], in_=dbc36[4:36, :])
            nc.sync.dma_start(out=bdr[:, L:TS], in_=zeros32[:, :])
            xs_map[(b, br)] = xs
            dbc_map[(b, br)] = dbc36
            bc_dram[(b, br)] = bdr

        # ---- D2a sweep: dt logits (PE) + e^z (Exp set), all units ----
        dt_map = {}
        for b, br in units():
            dbc36 = dbc_map[(b, br)]
            dtps = mm_ps.tile([128, L], f32, tag="mm")
            dw = c4[:, br * 128:(br + 1) * 128]
            nc.tensor.matmul(dtps[:, 0:512], lhsT=dw, rhs=dbc36[0:4, 0:512],
                             start=True, stop=True)
            nc.tensor.matmul(dtps[:, 512:768], lhsT=dw, rhs=dbc36[0:4, 512:768],
                             start=True, stop=True)
            dtk = dtp.tile([128, L], bf16, tag="dt")
            nc.scalar.activation(dtk[:, :], dtps[:, :], AF.Exp,
                                 bias=c128f[:, PF_DTB + br:PF_DTB + br + 1])
            dt_map[(b, br)] = dtk
        # ---- D2b sweep: softplus ln(1 + e^z) in-place (Ln set), all units ----
        for b, br in units():
            dtk = dt_map[(b, br)]
            nc.scalar.activation(dtk[:, :], dtk[:, :], AF.Ln,
                                 bias=c128f[:, 75:76])
        # ---- D2c sweep: dA = exp(A_s*dt) (Exp set) + u = dt*xs, all units ----
        dA_map = {}
        u_map = {}
        for b, br in units():
            xs = xs_map[(b, br)]
            dtk = dt_map[(b, br)]
            Ac = c128f[:, PF_A + br * 16: PF_A + (br + 1) * 16]
            halves = []
            for hf in range(2):
                dA = dAp.tile([128, SH, TS], bf16, tag="dA")
                nc.vector.memset(dA[:, :, L:TS], 0.0)
                for sj in range(SH):
                    si = hf * SH + sj
                    nc.scalar.activation(dA[:, sj, 0:L], dtk[:, :], AF.Exp,
                                         scale=Ac[:, si:si + 1])
                halves.append(dA)
            dA_map[(b, br)] = halves
            u = up.tile([128, L], bf16, tag="u")
            nc.vector.tensor_tensor(out=u[:, :], in0=dtk[:, :], in1=xs[:, :],
                                    op=AL.mult)
            u_map[(b, br)] = u

        # ---- D3 sweep: broadcast, dBu, scan, y reduce, gate ----
        y_sum = {}
        for b in range(BL):
            yst = ysp.tile([128, L], bf16, tag="ysum", name=f"ysum{b}")
            y_sum[b] = yst

        tree_eng = nc.gpsimd if USE_GPSIMD_TREE else nc.vector

        for b, br in units():
            img_i = 0 if br < 2 else 1
            xs = xs_map[(b, br)]
            u = u_map[(b, br)]
            dA_a, dA_b = dA_map[(b, br)]
            bdr = bc_dram[(b, br)]
            uap = u[:, :]
            u_bc = bass.AP(tensor=uap.tensor, offset=uap.offset,
                           ap=[list(uap.ap[0]), [0, SH], list(uap.ap[-1])])
            # B/C broadcast: DRAM bounce rows (TS-wide, breaks included)
            # flattened, partition-stride 0, in 8-state halves
            srcf = bdr[:, :].rearrange("a t -> (a t)")
            st0 = list(srcf.ap[-1])[0]

            def bcast_half(row0):
                t = bcp.tile([128, SH, TS], bf16, tag="bc")
                sap = bass.AP(tensor=srcf.tensor, offset=srcf.offset + row0 * TS,
                              ap=[[0, 128], [st0, SH * TS]])
                nc.sync.dma_start(out=t[:, :, :], in_=sap)
                return t

            hs = []
            for hf, dA in ((0, dA_a), (1, dA_b)):
                bbc = bcast_half(hf * SH)
                # dBu = u * B in place on the broadcast tile (breaks stay 0)
                nc.vector.tensor_tensor(out=bbc[:, :, 0:L], in0=u_bc,
                                        in1=bbc[:, :, 0:L], op=AL.mult)
                h = hp.tile([128, SH, TS], bf16, tag="h")
                nc.vector.tensor_tensor_scan(
                    out=h[:, :, :].rearrange("p s t -> p (s t)"),
                    data0=dA[:, :, :].rearrange("p s t -> p (s t)"),
                    data1=bbc[:, :, :].rearrange("p s t -> p (s t)"),
                    initial=0.0, op0=AL.mult, op1=AL.add)
                hs.append(h)
            # y = sum_s h*C: in-place mul per half + tree reduce
            for hf, h in ((0, hs[0]), (1, hs[1])):
                cbc = bcast_half(16 + hf * SH)
                nc.vector.tensor_tensor(out=h[:, :, 0:L], in0=h[:, :, 0:L],
                                        in1=cbc[:, :, 0:L], op=AL.mult)
            h0 = hs[0]
            tree_eng.tensor_tensor(out=h0[:, :, 0:L], in0=h0[:, :, 0:L],
                                   in1=hs[1][:, :, 0:L], op=AL.add)
            tree_eng.tensor_tensor(out=h0[:, 0:4, 0:L], in0=h0[:, 0:4, 0:L],
                                   in1=h0[:, 4:8, 0:L], op=AL.add)
            nc.vector.tensor_tensor(out=h0[:, 0:2, 0:L], in0=h0[:, 0:2, 0:L],
                                    in1=h0[:, 2:4, 0:L], op=AL.add)
            yb = seqp.tile([128, L], bf16, tag="yb")
            nc.vector.tensor_tensor(out=yb[:, :], in0=h0[:, 0, 0:L],
                                    in1=h0[:, 1, 0:L], op=AL.add)
            # gate + accumulate (in-place on yb)
            nc.vector.scalar_tensor_tensor(
                out=yb[:, :], in0=xs[:, :],
                scalar=c128f[:, PF_D + br:PF_D + br + 1],
                in1=yb[:, :], op0=AL.mult, op1=AL.add)
            zt = sz[(img_i, b)]
            if br == 0:
                nc.vector.tensor_tensor(out=y_sum[b][:, :], in0=yb[:, :],
                                        in1=zt[:, :], op=AL.mult)
            else:
                zin = rev(zt[:, :]) if br == 1 else zt[:, :]
                nc.vector.tensor_tensor(out=yb[:, :], in0=yb[:, :], in1=zin,
                                        op=AL.mult)
                t2in = rev(yb[:, :]) if br == 1 else yb[:, :]
                nc.vector.tensor_tensor(out=y_sum[b][:, :], in0=y_sum[b][:, :],
                                        in1=t2in, op=AL.add)

        # ---- final head ----
        att_sb = outp.tile([1, BL * L], bf16, tag="big")
        vcol = vcol_b[:, :]
        for b in range(BL):
            lg = mm_ps.tile([1, L], f32, tag="mm")
            nc.tensor.matmul(lg[:, 0:512], lhsT=vcol, rhs=y_sum[b][:, 0:512],
                             start=True, stop=True)
            nc.tensor.matmul(lg[:, 512:768], lhsT=vcol, rhs=y_sum[b][:, 512:768],
                             start=True, stop=True)
            nc.scalar.activation(att_sb[:, b * L:(b + 1) * L], lg[:, :], AF.Sigmoid,
                                 scale=0.5, bias=cmisc[0:1, 0:1])
        nc.sync.dma_start(out=att_out, in_=att_sb[:, :])

    nc.compile()
    return nc


def _pack_params(inputs):
    import ml_dtypes
    gi = lambda k: np.asarray(inputs[k], dtype=np.float32)

    p128f = np.zeros((128, PF_NCOL), np.float32)
    p128b = np.zeros((128, 108), np.float32)
    tags = ("f", "b", "s")
    for t, tag in enumerate(tags):
        p128f[:, PF_A + t * 16: PF_A + 16 + t * 16] = -np.exp(gi("A_log_" + tag))
        p128f[:, PF_CW + t * 4: PF_CW + 4 + t * 4] = gi("conv_w_" + tag)
        p128f[:, PF_CB + t] = gi("conv_b_" + tag)
        p128f[:, PF_DTB + t] = gi("dtproj_b_" + tag)
        p128f[:, PF_D + t] = gi("D_" + tag)
        p128b[:, t * 36:(t + 1) * 36] = gi("xproj_w_" + tag).T
    p128f[:, PF_V] = gi("out_proj_w").T @ gi("post_w")[0]
    p128f[:, 74] = LN_EPS
    p128f[:, 75] = 1.0
    ln_g, ln_b = gi("ln_g"), gi("ln_b")
    w1t = gi("in_proj_w").T
    w2t = gi("in_proj_s_w").T
    b1 = ln_b @ w1t
    b2 = ln_b @ w2t
    p128f[:, PF_BE + 0] = b1[0:128]
    p128f[:, PF_BE + 1] = b1[128:256]
    p128f[:, PF_BE + 2] = b2[0:128]
    p128f[:, PF_BE + 3] = b2[128:256]

    p17f = np.zeros((17, 64), np.float32)
    p17f[0:16] = gi("pre_w").T / 64.0
    p17f[16] = gi("pre_b")

    p4b = np.zeros((4, 384), np.float32)
    for t, tag in enumerate(tags):
        p4b[:, t * 128:(t + 1) * 128] = gi("dtproj_w_" + tag).T

    p64b = np.zeros((64, 512), np.float32)
    p64b[:, 0:256] = w1t * ln_g[:, None]
    p64b[:, 256:512] = w2t * ln_g[:, None]

    miscf = np.zeros((1, 2), np.float32)
    miscf[0, 0] = 0.5 * float(gi("post_b").reshape(-1)[0])

    bf = ml_dtypes.bfloat16
    return {
        "p128f": p128f,
        "p128b": p128b.astype(bf),
        "p17f": p17f.astype(bf),
        "p4b": p4b.astype(bf),
        "p64b": p64b.astype(bf),
        "miscf": miscf,
    }


def get_nc():
    if "nc" not in _cached:
        _cached["nc"] = _build_nc()
    return _cached["nc"]


def make_in_maps(inputs):
    params = _pack_params(inputs)
    img1 = np.ascontiguousarray(np.asarray(inputs["img1_features"], np.float32))
    img2 = np.ascontiguousarray(np.asarray(inputs["img2_features"], np.float32))
    in_maps = []
    for c in range(NCORES):
        m = dict(params)
        m["img1"] = np.ascontiguousarray(img1[c * BL:(c + 1) * BL])
        m["img2"] = np.ascontiguousarray(img2[c * BL:(c + 1) * BL])
        in_maps.append(m)
    return in_maps


def kernel(**inputs):
    from concourse.bass_utils import run_bass_kernel_spmd

    nc = get_nc()
    in_maps = make_in_maps(inputs)
    res = run_bass_kernel_spmd(nc, in_maps, core_ids=list(range(NCORES)))
    outs = [np.asarray(r["att"], dtype=np.float32).reshape(BL, L)
            for r in res.results]
    att = np.concatenate(outs, axis=0) + 1e-6
    return att.reshape(B, C, 1, 1).astype(np.float32)
